# revision 1
# baseline (speedup 1.0000x reference)
"""Trainium2 Bass kernel for nn_DiWeightedGCNLayer (8-core SPMD), v2.

Math (per reference):
    h   = LayerNorm(x) * gamma + beta
    m   = h @ W.T + b
    msg = m[src] * w
    out = segment_sum(msg, dst) / max(segment_sum(w, dst), 1) * dst_scale
    y   = x + gelu(out)

v2 architecture (vs v1's one indirect_dma_start per 128 edges, which
serialized on the Pool engine at ~1us SWDGE desc-gen per gather):

  * Linearity: W is applied AFTER aggregation.  Phase 1 only computes
    h = LN(x) (gamma/beta folded into W2 = gamma[:,None]*W.T on host) and
    stores it bf16 to DRAM in partition-major layout h2[r], r = (n%128)*nt
    + n//128, so phase-1 stores are 2KB-contiguous per partition.
  * Phase 2 uses bulk InstDMAGatherAnt (gpsimd.dma_gather): ONE Pool
    instruction gathers all edges of a 4-chunk group (~9k rows), so the
    994ns SWDGE fixed cost is paid ~26x per core instead of ~830x.
    Gather indices are int16, so the node table is split at row 32768
    into a lo and a hi gather per group (edges pre-sorted by half on
    host; each (chunk, half) segment padded to whole 128-edge blocks
    with idx=0 / w=0).
  * Scatter-add per dst chunk via one-hot matmul accumulating the
    TRANSPOSED aggregate: aggT[d,v] += matmul(lhsT=msg_block[e,d],
    rhs=onehot_w[e,v]); then out[v,d'] = matmul(lhsT=aggT_bf16, rhs=W2)
    once per chunk.
  * deg = segment_sum(w, dst) only feeds max(deg,1); host precomputes
    scl[v] = dst_scale[v]/max(deg[v],1) (and the c = beta@W.T + b term's
    deg*scl coefficient when c != 0), removing the per-block degree
    matmul entirely.

Hardware notes (learned in v1 + this session):
  - dma_gather idxs are int16, laid out [16, n/16] (idx i at partition
    i%16, col i//16) and replicated 8x down the 128 partitions; all 128
    partitions are bounds-checked.  Gathered row i lands at partition
    i%128, block i//128 of the [128, n/128, elem] SBUF dst.
  - Tile framework tracks DRAM tensor deps (verified: a gather reading
    h2 gets a sync dep on the h2-writing DMA).
  - GPSIMD needs load_library(mlp) once for InstDMAGatherAnt ucode.
"""

import contextlib
import numpy as np
import ml_dtypes

import concourse.bass as bass
import concourse.bacc as bacc
import concourse.tile as tile
import concourse.mybir as mybir
from concourse.bass_utils import run_bass_kernel_spmd
from concourse.library_config import mlp

F32 = mybir.dt.float32
BF16 = mybir.dt.bfloat16
I16 = mybir.dt.int16
AF = mybir.ActivationFunctionType
OP = mybir.AluOpType

D = 128
P = 128
LN_EPS = 1e-5
HALF = 32768  # int16 index ceiling for dma_gather


def build_program(n_pad, nch, include_c, b_lo, b_hi, loop_n=1, gc=4, g1=8,
                  msg_bufs=3, oh_bufs=8, psa_bufs=5, pso_bufs=2,
                  af_gelu=True, do_p1=True, do_p2=True, do_gather=True,
                  do_compute=True, gb=8, use_oh=True, nsub=1):
    """One-core SPMD program.

    n_pad: padded node count (multiple of 128); nch: dst chunks per core;
    b_lo/b_hi: per-chunk block counts for the lo/hi gathers (tuples, max
    over cores so the program is core-independent); gc: chunks per gather
    group; g1: node-row tiles per phase-1 iteration.
    """
    nt = n_pad // P
    b_lo = list(b_lo)
    b_hi = list(b_hi)
    groups = [list(range(g0, min(g0 + gc, nch))) for g0 in range(0, nch, gc)]
    # per-group widths (blocks) and per-chunk block starts within the group
    gWL = [sum(b_lo[c] for c in g) for g in groups]
    gWH = [sum(b_hi[c] for c in g) for g in groups]
    MW = max(wl + wh for wl, wh in zip(gWL, gWH))
    IW = sum(8 * (wl + wh) for wl, wh in zip(gWL, gWH))
    RW = sum(2 * (wl + wh) for wl, wh in zip(gWL, gWH))

    nc = bacc.Bacc(num_swdge_queues=4, dynamic_dma_scratch_size=32768)

    x_ext = nc.declare_dram_parameter("x", [n_pad, D], F32, isOutput=False)
    w2_ext = nc.declare_dram_parameter("w2", [D, D], BF16, isOutput=False)
    iota_ext = nc.declare_dram_parameter("iota", [P, P], BF16, isOutput=False)
    idx_ext = nc.declare_dram_parameter("idx16", [P, IW], I16, isOutput=False)
    if use_oh:
        TB = RW // 2
        oh_ext = nc.declare_dram_parameter("oht", [P, TB * P], BF16,
                                           isOutput=False)
    else:
        relw_ext = nc.declare_dram_parameter("relw", [P, RW], F32,
                                             isOutput=False)
    scl_ext = nc.declare_dram_parameter("scl", [P, nch], F32, isOutput=False)
    xres_ext = nc.declare_dram_parameter("xres", [P, nch * D], F32,
                                         isOutput=False)
    if include_c:
        cb_ext = nc.declare_dram_parameter("cb", [P, D], F32, isOutput=False)
        csc_ext = nc.declare_dram_parameter("csc", [P, nch], F32,
                                            isOutput=False)
    y_ext = nc.declare_dram_parameter("y", [nch * P, D], F32, isOutput=True)

    # nsub>1: that many independent h2 buffers; the loop body runs nsub full
    # applications so phase 1 of app k+1 overlaps phase 2 of app k (phase 2
    # only depends on its own h2 buffer).
    h2_drams = [nc.dram_tensor(f"h2_{s}", [n_pad, D], BF16)
                for s in range(nsub)]

    with tile.TileContext(nc) as tc:
        with (
            tc.tile_pool(name="const", bufs=1) as const,
            tc.tile_pool(name="xp", bufs=3) as xp,
            tc.tile_pool(name="stats", bufs=3) as sp,
            tc.tile_pool(name="small", bufs=4) as smp,
            tc.tile_pool(name="hp", bufs=3) as hp,
            tc.tile_pool(name="meta", bufs=3) as metp,
            tc.tile_pool(name="msg", bufs=msg_bufs) as msgp,
            tc.tile_pool(name="oh", bufs=oh_bufs) as ohp,
            tc.tile_pool(name="ep", bufs=4) as epp,
            tc.tile_pool(name="yt", bufs=3) as ytp,
            tc.tile_pool(name="ps_a", bufs=psa_bufs, space="PSUM") as psa,
            tc.tile_pool(name="ps_o", bufs=pso_bufs, space="PSUM") as pso,
        ):
            nc.gpsimd.load_library(mlp)

            # --- constants (outside the benchmark loop) ---
            w2_t = const.tile([D, D], BF16)
            nc.sync.dma_start(out=w2_t[:], in_=w2_ext[:, :])
            iota_t = const.tile([P, P], BF16)
            nc.sync.dma_start(out=iota_t[:], in_=iota_ext[:, :])
            eps_t = const.tile([P, 1], F32)
            nc.vector.memset(eps_t[:], LN_EPS)
            scl_t = const.tile([P, nch], F32)
            nc.sync.dma_start(out=scl_t[:], in_=scl_ext[:, :])
            xres_t = const.tile([P, nch * D], F32)
            nc.sync.dma_start(out=xres_t[:], in_=xres_ext[:, :])
            if include_c:
                cb_t = const.tile([P, D], F32)
                nc.sync.dma_start(out=cb_t[:], in_=cb_ext[:, :])
                csc_t = const.tile([P, nch], F32)
                nc.sync.dma_start(out=csc_t[:], in_=csc_ext[:, :])

            def one_app(h2_dram):
                h2w = h2_dram[:, :].rearrange("(p t) d -> p t d", p=P)
                # --- phase 1: h = LN(x), all nodes, bf16 to h2 (partition-
                # major: node n -> row (n%128)*nt + n//128) ---
                G = g1
                for t0 in range(0, nt if do_p1 else 0, G):
                    gn = min(G, nt - t0)
                    xt = xp.tile([P, G, D], F32)
                    x_src = x_ext[t0 * P:(t0 + gn) * P, :].rearrange(
                        "(j p) d -> p j d", p=P)
                    nc.sync.dma_start(out=xt[:, :gn, :], in_=x_src)
                    # NB: multi-sample (3D-out) bn_stats breaks tile dep
                    # tracking (bn_aggr scheduled before it) — keep 2D.
                    st = sp.tile([P, 6 * G], F32, tag="st")
                    mv = sp.tile([P, 2 * G], F32, tag="mv")
                    for j in range(gn):
                        nc.vector.bn_stats(out=st[:, 6 * j:6 * j + 6],
                                           in_=xt[:, j, :])
                        nc.vector.bn_aggr(out=mv[:, 2 * j:2 * j + 2],
                                          in_=st[:, 6 * j:6 * j + 6])
                    mv3 = mv[:].rearrange("p (g two) -> p g two", two=2)
                    sd = smp.tile([P, G, 1], F32, tag="sd")
                    nc.scalar.activation(out=sd[:, :gn, :],
                                         in_=mv3[:, :gn, 1:2],
                                         func=AF.Sqrt, bias=eps_t[:, :],
                                         scale=1.0)
                    rstd = smp.tile([P, G, 1], F32, tag="rstd")
                    nc.vector.reciprocal(out=rstd[:, :gn, :],
                                         in_=sd[:, :gn, :])
                    # nmu = -mean * rstd  (one fused DVE op)
                    nmu = smp.tile([P, G, 1], F32, tag="nmu")
                    nc.vector.scalar_tensor_tensor(
                        out=nmu[:, :gn, :], in0=mv3[:, :gn, 0:1], scalar=-1.0,
                        in1=rstd[:, :gn, :], op0=OP.mult, op1=OP.mult)
                    h4 = hp.tile([P, G, D], BF16)
                    for j in range(gn):
                        nc.scalar.activation(out=h4[:, j, :], in_=xt[:, j, :],
                                             func=AF.Identity,
                                             bias=nmu[:, j, :],
                                             scale=rstd[:, j, :])
                    nc.sync.dma_start(out=h2w[:, t0:t0 + gn, :],
                                      in_=h4[:, :gn, :])

                # --- phase 2: per group, bulk-gather then one-hot matmul ---
                ioff = 0
                roff = 0
                for gi, g in enumerate(groups if do_p2 else []):
                    WL, WH = gWL[gi], gWH[gi]
                    W = WL + WH
                    # block-start of each chunk's lo/hi section
                    los, his = [], []
                    a = 0
                    for c in g:
                        los.append(a)
                        a += b_lo[c]
                    a = 0
                    for c in g:
                        his.append(a)
                        a += b_hi[c]

                    idxt = metp.tile([P, 8 * MW], I16, tag="idx")
                    nc.sync.dma_start(out=idxt[:, :8 * W],
                                      in_=idx_ext[:, ioff:ioff + 8 * W])
                    if use_oh:
                        ohgt = ohp.tile([P, MW, P], BF16, tag="ohg",
                                        bufs=msg_bufs)
                        nc.sync.dma_start(
                            out=ohgt[:, :W, :],
                            in_=oh_ext[:, (roff // 2) * P:
                                       (roff // 2 + W) * P].rearrange(
                                "p (b v) -> p b v", v=P))
                    else:
                        relwt = metp.tile([P, 2 * MW], F32, tag="relw")
                        nc.sync.dma_start(out=relwt[:, :2 * W],
                                          in_=relw_ext[:, roff:roff + 2 * W])
                        r_t = relwt[:, 0:W]
                        w_t = relwt[:, W:2 * W]
                    ioff += 8 * W
                    roff += 2 * W

                    # ucode SWDGE ring caps one gather at 1024 descriptors
                    # (measured on HW: 1024 ok, 1280 fails) -> <=8 blocks per
                    # dma_gather instruction.
                    msgt = msgp.tile([P, MW, D], BF16, tag="msg")
                    qn = 2 * gi
                    GB = gb
                    bounds = []
                    for b0 in range(0, WL, GB):
                        bounds.append((b0, min(b0 + GB, WL), 0))
                    for b0 in range(WL, W, GB):
                        bounds.append((b0, min(b0 + GB, W), HALF))
                    if do_gather:
                        for (b0, b1, base) in bounds:
                            src = (h2_dram[0:HALF, :] if base == 0
                                   else h2_dram[HALF:n_pad, :])
                            nc.gpsimd.dma_gather(
                                msgt[:, b0:b1, :], src,
                                idxt[:, b0 * 8:b1 * 8], (b1 - b0) * P,
                                (b1 - b0) * P, D, queue_num=qn % 4)
                            qn += 1

                    def msg_block(b):
                        return msgt[:, b, :]

                    ytg = ytp.tile([P, gc, D], F32, tag="ytg")
                    if do_gather and not do_compute:
                        # light consumer so SWDGE ring reclaim has
                        # back-pressure (gather-only timing variant)
                        nc.vector.tensor_copy(out=ytg[:, 0, :],
                                              in_=msgt[:, W - 1, :])
                    for jc, c in enumerate(g if do_compute else []):
                        blocks = (
                            list(range(los[jc], los[jc] + b_lo[c])) +
                            list(range(WL + his[jc], WL + his[jc] + b_hi[c])))
                        agg = psa.tile([P, D], F32)
                        for k, b in enumerate(blocks):
                            if use_oh:
                                oh = ohgt[:, b, :]
                            else:
                                oht = ohp.tile([P, P], BF16)
                                nc.vector.tensor_scalar(
                                    out=oht[:], in0=iota_t[:],
                                    scalar1=r_t[:, b:b + 1],
                                    scalar2=w_t[:, b:b + 1],
                                    op0=OP.is_equal, op1=OP.mult)
                                oh = oht[:]
                            nc.tensor.matmul(out=agg[:], lhsT=msg_block(b),
                                             rhs=oh, start=(k == 0),
                                             stop=(k == len(blocks) - 1))
                        aggm = smp.tile([P, D], BF16, tag="aggm")
                        nc.scalar.copy(out=aggm[:], in_=agg[:])
                        outp = pso.tile([P, D], F32)
                        nc.tensor.matmul(out=outp[:], lhsT=aggm[:],
                                         rhs=w2_t[:], start=True, stop=True)
                        sc = epp.tile([P, D], F32, tag="sc")
                        nc.vector.tensor_scalar(out=sc[:], in0=outp[:],
                                                scalar1=scl_t[:, c:c + 1],
                                                scalar2=None, op0=OP.mult)
                        if include_c:
                            nc.vector.scalar_tensor_tensor(
                                out=sc[:], in0=cb_t[:],
                                scalar=csc_t[:, c:c + 1], in1=sc[:],
                                op0=OP.mult, op1=OP.add)
                        gl = epp.tile([P, D], F32, tag="gl")
                        if af_gelu:
                            nc.scalar.activation(out=gl[:], in_=sc[:],
                                                 func=AF.Gelu)
                        else:
                            # tanh-gelu composition (CoreSim lacks the Gelu
                            # table); sim-validation only
                            sq = epp.tile([P, D], F32, tag="sq")
                            nc.vector.tensor_mul(out=sq[:], in0=sc[:],
                                                 in1=sc[:])
                            cu = epp.tile([P, D], F32, tag="cu")
                            nc.vector.tensor_mul(out=cu[:], in0=sq[:],
                                                 in1=sc[:])
                            u = epp.tile([P, D], F32, tag="u")
                            nc.vector.scalar_tensor_tensor(
                                out=u[:], in0=cu[:], scalar=0.044715,
                                in1=sc[:], op0=OP.mult, op1=OP.add)
                            v = epp.tile([P, D], F32, tag="v")
                            nc.scalar.activation(
                                out=v[:], in_=u[:], func=AF.Tanh,
                                scale=0.7978845608028654)
                            w1 = epp.tile([P, D], F32, tag="w1")
                            nc.vector.tensor_mul(out=w1[:], in0=sc[:],
                                                 in1=v[:])
                            nc.vector.tensor_add(out=w1[:], in0=w1[:],
                                                 in1=sc[:])
                            nc.vector.tensor_scalar(
                                out=gl[:], in0=w1[:], scalar1=0.5,
                                scalar2=None, op0=OP.mult)
                        nc.vector.tensor_add(
                            out=ytg[:, jc, :], in0=gl[:],
                            in1=xres_t[:, c * D:(c + 1) * D])
                    if do_compute:
                        y_dst = y_ext[g[0] * P:(g[0] + len(g)) * P, :]\
                            .rearrange("(j p) d -> p j d", p=P)
                        nc.sync.dma_start(out=y_dst, in_=ytg[:, :len(g), :])

            loop_ctx = (tc.For_i(0, loop_n, 1) if loop_n > 1
                        else contextlib.nullcontext())
            with loop_ctx:
                for _s in range(nsub):
                    one_app(h2_drams[_s])

    return nc


def prepare_inputs(x, gamma, beta, W, b, edge_index, edge_weight, dst_scale,
                   n_cores, gc=4):
    """Host-side prep: sort edges by (dst-chunk, idx-half), pad each
    (core, chunk, half) segment to whole 128-edge blocks, build int16
    gather-index tables + rel/weight tables + scl = dst_scale/max(deg,1).
    """
    N = x.shape[0]
    R = n_cores
    npc = (N + R - 1) // R
    nch = (npc + P - 1) // P
    npc_pad = nch * P
    n_pad = (((R - 1) * npc + npc_pad + P - 1) // P) * P
    nt = n_pad // P

    src = np.ascontiguousarray(edge_index[0]).astype(np.int64)
    dst = np.ascontiguousarray(edge_index[1]).astype(np.int64)
    w = edge_weight.astype(np.float32)
    E = src.shape[0]

    # deg -> scl on host (feeds only max(deg,1) normalization)
    deg = np.zeros(N, np.float32)
    np.add.at(deg, dst, w)
    scl_full = dst_scale.astype(np.float32) / np.maximum(deg, 1.0)

    core_id = np.minimum(dst // npc, R - 1)
    local = dst - core_id * npc
    chunk = local // P
    rel = (local % P).astype(np.float32)
    gidx = (src % P) * nt + src // P  # h2 row index
    half = (gidx >= HALF).astype(np.int64)

    key = (core_id * nch + chunk) * 2 + half
    order = np.argsort(key, kind="stable")
    key_s = key[order]
    gidx_s = gidx[order]
    rel_s = rel[order]
    w_s = w[order]

    nseg = R * nch * 2
    cnt = np.bincount(key_s, minlength=nseg).reshape(R, nch, 2)
    blk = -(-cnt // P)  # ceil
    b_lo = np.maximum(blk[:, :, 0].max(axis=0), 1)  # >=1 so agg is defined
    b_hi = blk[:, :, 1].max(axis=0)
    b_lo_t = tuple(int(v) for v in b_lo)
    b_hi_t = tuple(int(v) for v in b_hi)

    groups = [list(range(g0, min(g0 + gc, nch))) for g0 in range(0, nch, gc)]
    gWL = [sum(b_lo[c] for c in g) for g in groups]
    gWH = [sum(b_hi[c] for c in g) for g in groups]
    IW = sum(8 * (wl + wh) for wl, wh in zip(gWL, gWH))
    RW = sum(2 * (wl + wh) for wl, wh in zip(gWL, gWH))

    # per chunk: group id, block offset of its lo/hi sections within the
    # group's gather position space, and the group's column offsets
    grp_of = np.empty(nch, np.int64)
    lo_start = np.empty(nch, np.int64)  # block start within lo gather
    hi_start = np.empty(nch, np.int64)  # block start within hi gather
    g_icol = np.empty(len(groups), np.int64)  # idx_ext column offset
    g_rcol = np.empty(len(groups), np.int64)  # relw_ext column offset
    io, ro = 0, 0
    for gi, g in enumerate(groups):
        g_icol[gi] = io
        g_rcol[gi] = ro
        a = 0
        for c in g:
            grp_of[c] = gi
            lo_start[c] = a
            a += b_lo[c]
        a = 0
        for c in g:
            hi_start[c] = a
            a += b_hi[c]
        io += 8 * (gWL[gi] + gWH[gi])
        ro += 2 * (gWL[gi] + gWH[gi])

    seg_starts = np.searchsorted(key_s, np.arange(nseg + 1))
    pos = np.arange(E) - seg_starts[key_s]

    core_s = key_s // (2 * nch)
    ch_s = (key_s // 2) % nch
    half_s = key_s % 2
    gi_s = grp_of[ch_s]
    # gather position i within the (lo|hi) gather of the chunk's group
    gpos = np.where(half_s == 0, lo_start[ch_s], hi_start[ch_s]) * P + pos
    # block column within the group's msg tile (lo blocks, then hi blocks)
    bcol = np.where(half_s == 0, gpos // P,
                    np.asarray(gWL, np.int64)[gi_s] + gpos // P)

    x_pad = np.zeros((n_pad, D), np.float32)
    x_pad[:N] = np.asarray(x, np.float32)

    W2 = (np.asarray(W).T.astype(np.float32)
          * np.asarray(gamma, np.float32)[:, None])
    W2 = np.ascontiguousarray(W2).astype(ml_dtypes.bfloat16)
    c_row = (np.asarray(beta, np.float32) @ np.asarray(W, np.float32).T
             + np.asarray(b, np.float32))
    include_c = bool(np.any(c_row != 0.0))
    cb = np.ascontiguousarray(
        np.broadcast_to(c_row, (P, D))).astype(np.float32)

    iota = np.ascontiguousarray(np.broadcast_to(
        np.arange(P, dtype=np.float32), (P, P))).astype(ml_dtypes.bfloat16)

    in_maps = []
    for r in range(R):
        m = core_s == r
        gi_r = gi_s[m]
        half_r = half_s[m]
        gpos_r = gpos[m]
        bcol_r = bcol[m]
        gidx_r = gidx_s[m]
        rel_r = rel_s[m]
        w_r = w_s[m]

        # int16 idx tables: per group [lo idxs | hi idxs], each wrapped
        # [16, n/16] then replicated to 128 partitions
        idx_cols = np.zeros((128, IW), np.int16)
        relw_cols = np.zeros((128, RW), np.float32)
        TB = RW // 2
        # dense one-hot tables: block column bcol, entry [e=gpos%128,
        # bcol*128 + rel] = w
        oh_cols = np.zeros((128, TB * P), ml_dtypes.bfloat16)
        for gi, g in enumerate(groups):
            WLg, WHg = gWL[gi], gWH[gi]
            Wg = WLg + WHg
            sel = (gi_r == gi)
            for hf, nblk, coff in ((0, WLg, 0), (1, WHg, WLg)):
                if nblk == 0:
                    continue
                nidx = nblk * P
                arr = np.zeros(nidx, np.int16)
                s2 = sel & (half_r == hf)
                vals = gidx_r[s2] - (HALF if hf else 0)
                arr[gpos_r[s2]] = vals.astype(np.int16)
                wrap = arr.reshape(nidx // 16, 16).T  # [16, nidx/16]
                col0 = g_icol[gi] + coff * 8
                idx_cols[:, col0:col0 + nidx // 16] = np.tile(wrap, (8, 1))
            # rel/w tables: [128, Wg] each, row = gpos%128, col = bcol
            rt = np.zeros((128, Wg), np.float32)
            wt = np.zeros((128, Wg), np.float32)
            rt[gpos_r[sel] % P, bcol_r[sel]] = rel_r[sel]
            wt[gpos_r[sel] % P, bcol_r[sel]] = w_r[sel]
            rc = g_rcol[gi]
            relw_cols[:, rc:rc + Wg] = rt
            relw_cols[:, rc + Wg:rc + 2 * Wg] = wt
            boff = rc // 2  # block-column offset of this group
            oh_cols[gpos_r[sel] % P,
                    (boff + bcol_r[sel]) * P + rel_r[sel].astype(np.int64)] \
                = w_r[sel].astype(ml_dtypes.bfloat16)

        lo = r * npc
        hi_n = min(N, lo + npc)
        scl_r = np.zeros(npc_pad, np.float32)
        scl_r[:hi_n - lo] = scl_full[lo:hi_n]
        sclt = np.ascontiguousarray(scl_r.reshape(nch, P).T)
        xr = np.zeros((npc_pad, D), np.float32)
        xr[:hi_n - lo] = x_pad[lo:hi_n]
        xres2 = np.ascontiguousarray(
            xr.reshape(nch, P, D).transpose(1, 0, 2).reshape(P, nch * D))

        mm = {
            "x": x_pad,
            "w2": W2,
            "iota": iota,
            "idx16": idx_cols,
            "relw": relw_cols,
            "oht": oh_cols,
            "scl": sclt,
            "xres": xres2,
        }
        if include_c:
            degsc = np.zeros(npc_pad, np.float32)
            degsc[:hi_n - lo] = (deg[lo:hi_n] * scl_full[lo:hi_n])
            mm["cb"] = cb
            mm["csc"] = np.ascontiguousarray(degsc.reshape(nch, P).T)
        in_maps.append(mm)

    geom = dict(n_pad=n_pad, nch=nch, include_c=include_c, b_lo=b_lo_t,
                b_hi=b_hi_t, gc=gc, npc=npc, npc_pad=npc_pad, N=N, R=R)
    return in_maps, geom


_PROGRAM_CACHE = {}


def kernel(x, gamma, beta, W, b, edge_index, num_nodes, edge_weight,
           dst_scale, n_cores=8, _collect=None):
    x = np.asarray(x)
    N = x.shape[0]
    in_maps, geom = prepare_inputs(
        np.asarray(x), np.asarray(gamma), np.asarray(beta), np.asarray(W),
        np.asarray(b), np.asarray(edge_index), np.asarray(edge_weight),
        np.asarray(dst_scale), n_cores)

    key = (geom["n_pad"], geom["nch"], geom["include_c"], geom["b_lo"],
           geom["b_hi"], geom["gc"])
    nc = _PROGRAM_CACHE.get(key)
    if nc is None:
        nc = build_program(n_pad=key[0], nch=key[1], include_c=key[2],
                           b_lo=key[3], b_hi=key[4], gc=key[5])
        nc.finalize()
        _PROGRAM_CACHE[key] = nc

    res = run_bass_kernel_spmd(nc, in_maps, list(range(n_cores)),
                               **(_collect.pop("kwargs") if _collect else {}))
    if _collect is not None:
        _collect["res"] = res

    y = np.empty((N, D), np.float32)
    npc = geom["npc"]
    for r in range(geom["R"]):
        lo = r * npc
        hi = min(N, lo + npc)
        y[lo:hi] = res.results[r]["y"][:hi - lo]
    return y

